# revision 10
# baseline (speedup 1.0000x reference)
"""ClassWeightedModalDownSampler Trainium2 kernel (packed exponent planes).

Problem: labels [4, 1024, 2048] int (values 0..19), class_weights [20] f32,
dsf=8.  Output modes [4, 128, 256]: per non-overlapping 8x8 patch, the
argmax over classes of (class histogram * class_weights), first-index
tie-break (jnp.argmax semantics).

Key idea: instead of 20 one-hot planes, build FIVE "packed exponent"
planes.  The host uploads u16 = (6*x + 127) << 7 (int16) = the bf16 bit
pattern of 2^(6x) = 64^x.  For plane a (classes 4a..4a+3), ONE fused DVE
op  (u16 min M_a) - 3072*a  clamps the exponent at class 4a+3 and
rebases, so the int16 bits, reinterpreted as bf16, equal

    64^(x-4a)        for x in {4a..4a+2}   (1, 64, 4096)
    64^3 = 262144    for x >= 4a+3         (clamped)
    2^(6(x-4a)) < 1  for x < 4a            (harmless dust, < 0.24/patch)

A ones-lhsT matmul sums each 8x8 patch: the fp32 PSUM value is the exact
4-digit base-64 number  n0 + 64*n1 + 4096*n2 + 262144*U3  where n_j are
class counts and U3 = #{x >= 4a+3} (a cumulative count).  Since
n0+n1+n2+U3 = 64, the sum is <= 2^24 and exact in fp32.  Digit peeling
(ACT truncs with round-nearest-safe biases + DVE remainder STTs)
recovers n/U; chain recovery n_{4a+3} = U3(a) - sum(digits(a+1)) runs as
a small bf16 matmul that directly emits E = 64*w_c*n_c - c for those
classes; E for the direct classes is a per-partition-scalar op.  Max
over classes + the baseline's (F+25)/64 decode give the argmax with
first-index tie-break.

Data parallel over 8 cores (64 patch rows each); per half (8 of 16
column chunks) the layout matches the old kernel: partition p = w mod
128 (16 patch-cols x 8 pixels), free n = r*512 + wcl*64 + prow.
"""

import numpy as np
import ml_dtypes

import concourse.bass as bass
import concourse.mybir as mybir
import concourse.tile as tile
from concourse import bacc
from concourse.bass_utils import run_bass_kernel_spmd

NCORES = 8
B, H, W = 4, 1024, 2048
DSF = 8
NCLS = 20
GH, GW = H // DSF, W // DSF  # 128, 256 output grid
ROWS = (B * H) // NCORES     # 512 label rows per core
PROWS = ROWS // DSF          # 64 patch rows per core
P = 128
WC = W // P                  # 16 column chunks of 128
HALVES = 2
WCH = WC // HALVES           # 8 chunks per half
FREE = WC * ROWS             # 8192
HFREE = FREE // HALVES       # 4096
NPL = 5                      # packed planes (4 classes each)

_DT = mybir.dt
_A = mybir.AluOpType
_AF = mybir.ActivationFunctionType

# Set by test.py to request a traced run.
TRACE = False
LAST_RESULTS = None


def _aux_arrays(class_weights: np.ndarray):
    """Host-built constants: stage-A/recovery lhsT, scalar APs, biases."""
    w = np.asarray(class_weights, dtype=np.float32)

    # stage-A lhsT: plane a sums 8-partition groups into M-slot a*16 + j.
    lhA = np.zeros((P, NPL * P), dtype=np.float32)
    for a in range(NPL):
        for p in range(P):
            lhA[p, a * P + a * 16 + p // 8] = 1.0
    lhA = lhA.astype(ml_dtypes.bfloat16)

    # recovery lhsT: E3[m = a*16+j] = 64*w[4a+3]*(d3(a) - sum_k d_k(a+1)) -
    # (4a+3).  Blocks k=0..3 contract the 80 digit partitions (K=80);
    # block 4 is the bias row (K=1, contracted against a ones tile).
    lhR = np.zeros((P, 5 * P), dtype=np.float32)
    for a in range(NPL):
        c = 4 * a + 3
        for j in range(16):
            m = a * 16 + j
            lhR[a * 16 + j, 3 * P + m] = 64.0 * w[c]
            if a + 1 < NPL:
                for k in range(4):
                    lhR[(a + 1) * 16 + j, k * P + m] = -64.0 * w[c]
            lhR[0, 4 * P + m] = -float(c)
    lhR = lhR.astype(ml_dtypes.bfloat16)

    # per-partition scalars for direct classes: E_jd = 64*w[4a+jd]*d - c
    wap = np.zeros((P, 3), dtype=np.float32)
    cap = np.zeros((P, 3), dtype=np.float32)
    for jd in range(3):
        for m in range(NPL * 16):
            a = m // 16
            wap[m, jd] = 64.0 * w[4 * a + jd]
            cap[m, jd] = float(4 * a + jd)

    # ACT biases: [-0.492 (t3/t2), -0.498 (d1), -0.375 (d0), 0.0]
    actb = np.zeros((P, 4), dtype=np.float32)
    actb[:, 0] = -0.492
    actb[:, 1] = -0.498
    actb[:, 2] = -0.375
    return lhA, lhR, wap, cap, actb


def _build():
    nc = bacc.Bacc(
        "TRN2",
        target_bir_lowering=False,
        debug=False,
        num_devices=NCORES,
    )
    u_d = nc.dram_tensor("u", [P, FREE], _DT.int16, kind="ExternalInput").ap()
    lha_d = nc.dram_tensor("lha", [P, NPL * P], _DT.bfloat16, kind="ExternalInput").ap()
    lhr_d = nc.dram_tensor("lhr", [P, 5 * P], _DT.bfloat16, kind="ExternalInput").ap()
    wap_d = nc.dram_tensor("wap", [P, 3], _DT.float32, kind="ExternalInput").ap()
    cap_d = nc.dram_tensor("cap", [P, 3], _DT.float32, kind="ExternalInput").ap()
    actb_d = nc.dram_tensor("actb", [P, 4], _DT.float32, kind="ExternalInput").ap()
    out_d = nc.dram_tensor("out", [16, HALVES * 512], _DT.int32, kind="ExternalOutput").ap()

    with tile.TileContext(nc) as tc:
        with (
            tc.tile_pool(name="const", bufs=1) as cpool,
            tc.tile_pool(name="u", bufs=1) as upool,
            tc.tile_pool(name="pk", bufs=2) as kpool,
            tc.tile_pool(name="psA", bufs=2, space="PSUM") as pApool,
            tc.tile_pool(name="psE", bufs=2, space="PSUM") as pEpool,
            tc.tile_pool(name="tail", bufs=2) as tpool,
            tc.tile_pool(name="outp", bufs=1) as outpool,
        ):
            # u chunks own the sync queue from t=0; consts ride SWDGE
            ut = upool.tile([P, FREE], _DT.int16)
            NCH = FREE // 2048
            for ch in range(NCH):
                nc.sync.dma_start(out=ut[:, ch * 2048:(ch + 1) * 2048],
                                  in_=u_d[:, ch * 2048:(ch + 1) * 2048])

            lhA = cpool.tile([P, NPL * P], _DT.bfloat16)
            nc.gpsimd.dma_start(out=lhA[:], in_=lha_d)
            lhR = cpool.tile([P, 5 * P], _DT.bfloat16)
            nc.gpsimd.dma_start(out=lhR[:], in_=lhr_d)
            wap = cpool.tile([P, 3], _DT.float32)
            nc.gpsimd.dma_start(out=wap[:], in_=wap_d)
            cap = cpool.tile([P, 3], _DT.float32)
            nc.gpsimd.dma_start(out=cap[:], in_=cap_d)
            actb = cpool.tile([P, 4], _DT.float32)
            nc.gpsimd.dma_start(out=actb[:], in_=actb_d)
            ones = cpool.tile([1, 512], _DT.bfloat16)
            nc.vector.memset(ones[:, :], 1.0)

            out_t = outpool.tile([16, HALVES * 512], _DT.int32)

            banks = []
            for hf in range(HALVES):
                base = hf * HFREE
                bank = pApool.tile([P, 512], _DT.float32, name=f"S{hf}", tag=f"S{hf}")
                banks.append(bank)
                for a in range(NPL):
                    pk = kpool.tile([P, HFREE], _DT.int16, name=f"pk{a}", tag=f"pk{a}")
                    nchunk = 2 if (hf == 0 and a == 0) else 1
                    for c in range(nchunk):
                        w0, w1 = c * HFREE // nchunk, (c + 1) * HFREE // nchunk
                        nc.vector.tensor_scalar(
                            out=pk[:, w0:w1],
                            in0=ut[:, base + w0:base + w1],
                            scalar1=float((145 + 24 * a) * 128),
                            scalar2=float(3072 * a),
                            op0=_A.min, op1=_A.subtract,
                        )
                        rhs = pk[:].bitcast(_DT.bfloat16)
                        for r in range(w0 // 512, w1 // 512):
                            nc.tensor.matmul(
                                bank[:, :],
                                lhA[:, a * P:(a + 1) * P],
                                rhs[:, r * 512:(r + 1) * 512],
                                start=(a == 0 and r == 0),
                                stop=(a == NPL - 1 and r == DSF - 1),
                            )

            for hf in range(HALVES):
                bank = banks[hf]
                hp = tc.high_priority() if hf == HALVES - 1 else None
                if hp is not None:
                    hp.__enter__()

                # digit peel, reading PSUM directly
                s80 = bank[0:80, :]
                t3 = tpool.tile([80, 512], _DT.int16, name="t3", tag="t3")
                nc.scalar.activation(t3[:], s80, _AF.Identity,
                                     bias=actb[0:80, 0:1], scale=1.0 / 262144)
                t2 = tpool.tile([80, 512], _DT.int16, name="t2", tag="t2")
                nc.scalar.activation(t2[:], s80, _AF.Identity,
                                     bias=actb[0:80, 0:1], scale=1.0 / 4096)
                d2b = tpool.tile([80, 512], _DT.bfloat16, name="d2b", tag="d2b")
                nc.vector.scalar_tensor_tensor(
                    out=d2b[:], in0=t3[:], scalar=-64.0, in1=t2[:],
                    op0=_A.mult, op1=_A.add)
                rem2 = tpool.tile([80, 512], _DT.float32, name="rem2", tag="rem2")
                nc.vector.scalar_tensor_tensor(
                    out=rem2[:], in0=t2[:], scalar=-4096.0, in1=s80,
                    op0=_A.mult, op1=_A.add)
                d1i = tpool.tile([80, 512], _DT.int16, name="d1i", tag="d1i")
                nc.scalar.activation(d1i[:], rem2[:], _AF.Identity,
                                     bias=actb[0:80, 1:2], scale=1.0 / 64)
                d0f = tpool.tile([80, 512], _DT.float32, name="d0f", tag="d0f")
                nc.vector.scalar_tensor_tensor(
                    out=d0f[:], in0=d1i[:], scalar=-64.0, in1=rem2[:],
                    op0=_A.mult, op1=_A.add)
                d0i = tpool.tile([80, 512], _DT.int16, name="d0i", tag="d0i")
                nc.scalar.activation(d0i[:], d0f[:], _AF.Identity,
                                     bias=actb[0:80, 2:3], scale=1.0)

                # bf16 digit tiles: d3b off-path on ACT, d1b/d0b on DVE 4x
                d3b = tpool.tile([80, 512], _DT.bfloat16, name="d3b", tag="d3b")
                nc.scalar.activation(d3b[:], t3[:], _AF.Identity,
                                     bias=actb[0:80, 3:4], scale=1.0)
                d1b = tpool.tile([80, 512], _DT.bfloat16, name="d1b", tag="d1b")
                nc.vector.tensor_scalar(out=d1b[:], in0=d1i[:], scalar1=0.0,
                                        scalar2=None, op0=_A.add)
                d0b = tpool.tile([80, 512], _DT.bfloat16, name="d0b", tag="d0b")
                nc.vector.tensor_scalar(out=d0b[:], in0=d0i[:], scalar1=0.0,
                                        scalar2=None, op0=_A.add)

                # E for chain classes: bias row + digits in reverse order so
                # the last-ready digit (d0b) is the final accumulate
                ps2 = pEpool.tile([P, 512], _DT.float32, name=f"E{hf}", tag=f"E{hf}")
                nc.tensor.matmul(
                    ps2[:, :], lhR[0:1, 4 * P:5 * P], ones[:, :],
                    start=True, stop=False,
                )
                dbs = (d0b, d1b, d2b, d3b)
                for k in (3, 2, 1, 0):
                    nc.tensor.matmul(
                        ps2[:, :], lhR[0:80, k * P:(k + 1) * P], dbs[k][:, :],
                        start=False, stop=(k == 0),
                    )

                # E for direct classes: per-partition scalars on GPSIMD
                es = []
                for jd, db in ((0, d0b), (1, d1b), (2, d2b)):
                    e = tpool.tile([80, 512], _DT.float32, name=f"e{jd}", tag=f"e{jd}")
                    nc.gpsimd.tensor_scalar(
                        out=e[:], in0=db[:],
                        scalar1=wap[0:80, jd:jd + 1], scalar2=cap[0:80, jd:jd + 1],
                        op0=_A.mult, op1=_A.subtract,
                    )
                    es.append(e)

                # tree max over the 4 E sources
                m01 = tpool.tile([80, 512], _DT.float32, name="m01", tag="m01")
                nc.vector.tensor_tensor(out=m01[:], in0=es[0][:], in1=es[1][:],
                                        op=_A.max)
                m23 = tpool.tile([80, 512], _DT.float32, name="m23", tag="m23")
                nc.vector.tensor_tensor(out=m23[:], in0=es[2][:], in1=ps2[0:80, :],
                                        op=_A.max)
                m3 = tpool.tile([96, 512], _DT.float32, name="m3", tag="m3")
                nc.vector.tensor_tensor(out=m3[0:80, :], in0=m01[:], in1=m23[:],
                                        op=_A.max)

                # fold the 5 plane-partitions (a*16+j) down to 16 (j)
                if hf == HALVES - 1:
                    # exposed tail: partition moves via DVE stream_shuffle
                    # (low latency) instead of DMA round-trips
                    idm = list(range(32))
                    upm = [16 + (i % 16) for i in range(32)]
                    sh1 = tpool.tile([32, 512], _DT.float32, name="sh1", tag="sh1")
                    nc.vector.stream_shuffle(sh1[:], m3[32:64, :], idm)
                    sh3 = tpool.tile([32, 512], _DT.float32, name="sh3", tag="sh3")
                    nc.vector.stream_shuffle(sh3[:], m3[64:96, :], idm)
                    f1 = tpool.tile([32, 512], _DT.float32, name="f1", tag="f1")
                    nc.vector.tensor_tensor(out=f1[:], in0=m3[0:32, :], in1=sh1[:],
                                            op=_A.max)
                    sh2 = tpool.tile([32, 512], _DT.float32, name="sh2", tag="sh2")
                    nc.vector.stream_shuffle(sh2[:], f1[:], upm)
                    f2 = tpool.tile([16, 512], _DT.float32, name="f2", tag="f2")
                    nc.vector.tensor_tensor(out=f2[:], in0=f1[0:16, :], in1=sh2[0:16, :],
                                            op=_A.max)
                    f3 = tpool.tile([16, 512], _DT.float32, name="f3", tag="f3")
                    nc.vector.tensor_tensor(out=f3[:], in0=f2[:], in1=sh3[0:16, :],
                                            op=_A.max)
                else:
                    t1 = tpool.tile([32, 512], _DT.float32, name="t1", tag="t1")
                    nc.sync.dma_start(out=t1[:], in_=m3[32:64, :])
                    t3f = tpool.tile([16, 512], _DT.float32, name="t3f", tag="t3f")
                    nc.sync.dma_start(out=t3f[:], in_=m3[64:80, :])
                    f1 = tpool.tile([32, 512], _DT.float32, name="f1", tag="f1")
                    nc.vector.tensor_tensor(out=f1[:], in0=m3[0:32, :], in1=t1[:],
                                            op=_A.max)
                    t2f = tpool.tile([16, 512], _DT.float32, name="t2f", tag="t2f")
                    nc.sync.dma_start(out=t2f[:], in_=f1[16:32, :])
                    f2 = tpool.tile([16, 512], _DT.float32, name="f2", tag="f2")
                    nc.vector.tensor_tensor(out=f2[:], in0=f1[0:16, :], in1=t2f[:],
                                            op=_A.max)
                    f3 = tpool.tile([16, 512], _DT.float32, name="f3", tag="f3")
                    nc.vector.tensor_tensor(out=f3[:], in0=f2[:], in1=t3f[:],
                                            op=_A.max)

                # decode: F = 64*w*n - c; W = cast((F + 25)/64); c* = 64W - F
                wi = tpool.tile([16, 512], _DT.int32, name="wi", tag="wi")
                wi_eng = nc.vector if hf == HALVES - 1 else nc.gpsimd
                wi_eng.tensor_scalar(
                    out=wi[:], in0=f3[:],
                    scalar1=25.0, scalar2=1.0 / 64.0,
                    op0=_A.add, op1=_A.mult,
                )
                nc.vector.scalar_tensor_tensor(
                    out=out_t[:, hf * 512:(hf + 1) * 512], in0=wi[:],
                    scalar=64.0, in1=f3[:],
                    op0=_A.mult, op1=_A.subtract,
                )
                nc.sync.dma_start(
                    out=out_d[:, hf * 512:(hf + 1) * 512],
                    in_=out_t[:, hf * 512:(hf + 1) * 512],
                )
                if hp is not None:
                    hp.__exit__(None, None, None)
    nc.finalize()
    return nc


_CACHED = None


def _get_nc():
    global _CACHED
    if _CACHED is None:
        _CACHED = _build()
    return _CACHED


def kernel(labels: np.ndarray, class_weights: np.ndarray, dsf) -> np.ndarray:
    global LAST_RESULTS
    dsf = int(np.asarray(dsf))
    assert dsf == DSF, f"kernel hardcodes dsf=8, got {dsf}"
    labels = np.asarray(labels)
    out_dtype = labels.dtype
    cw = np.asarray(class_weights, dtype=np.float32)

    # host prep: shard rows, encode u16 = (6x+127)<<7, transpose to
    # [p, hf, r, wcl, prow] (identical layout to the bf16 baseline)
    lab = labels.reshape(B * H, W).astype(np.int16)
    u_all = ((6 * lab + 127) << 7).astype(np.int16)
    lhA, lhR, wap, cap, actb = _aux_arrays(cw)
    in_maps = []
    for k in range(NCORES):
        shard = u_all[k * ROWS:(k + 1) * ROWS]                # [512, 2048]
        u = shard.reshape(PROWS, DSF, HALVES, WCH, P).transpose(4, 2, 1, 3, 0)
        u = np.ascontiguousarray(u).reshape(P, FREE)
        in_maps.append({
            "u": u,
            "lha": lhA,
            "lhr": lhR,
            "wap": wap,
            "cap": cap,
            "actb": actb,
        })

    nc = _get_nc()
    res = run_bass_kernel_spmd(
        nc, in_maps, core_ids=list(range(NCORES)), trace=TRACE,
    )
    LAST_RESULTS = res

    # unshard: core k out [16, 1024] int32; out[jj, hf*512 + wcl*64 + prow]
    # -> modes[patch_row = 64k + prow, j = (hf*8 + wcl)*16 + jj]
    modes = np.empty((B * GH, GW), dtype=np.int64)
    for k in range(NCORES):
        o = res.results[k]["out"].reshape(16, HALVES, WCH, PROWS)
        blk = o.transpose(3, 1, 2, 0).reshape(PROWS, WC * 16)
        modes[k * PROWS:(k + 1) * PROWS] = blk
    return modes.reshape(B, GH, GW).astype(out_dtype)


# revision 11
# speedup vs baseline: 1.2655x; 1.2655x over previous
"""ClassWeightedModalDownSampler Trainium2 kernel (packed exponent planes).

Problem: labels [4, 1024, 2048] int (values 0..19), class_weights [20] f32,
dsf=8.  Output modes [4, 128, 256]: per non-overlapping 8x8 patch, the
argmax over classes of (class histogram * class_weights), first-index
tie-break (jnp.argmax semantics).

Key idea: instead of 20 one-hot planes, build FIVE "packed exponent"
planes.  The host uploads u16 = (6*x + 127) << 7 (int16) = the bf16 bit
pattern of 2^(6x) = 64^x.  For plane a (classes 4a..4a+3), ONE fused DVE
op  (u16 min M_a) - 3072*a  clamps the exponent at class 4a+3 and
rebases, so the int16 bits, reinterpreted as bf16, equal

    64^(x-4a)        for x in {4a..4a+2}   (1, 64, 4096)
    64^3 = 262144    for x >= 4a+3         (clamped)
    2^(6(x-4a)) < 1  for x < 4a            (harmless dust, < 0.24/patch)

A ones-lhsT matmul sums each 8x8 patch: the fp32 PSUM value is the exact
4-digit base-64 number  n0 + 64*n1 + 4096*n2 + 262144*U3  where n_j are
class counts and U3 = #{x >= 4a+3} (a cumulative count).  Since
n0+n1+n2+U3 = 64, the sum is <= 2^24 and exact in fp32.  Digit peeling
(ACT truncs with round-nearest-safe biases + DVE remainder STTs)
recovers n/U; chain recovery n_{4a+3} = U3(a) - sum(digits(a+1)) runs as
a small bf16 matmul that directly emits E = 64*w_c*n_c - c for those
classes; E for the direct classes is a per-partition-scalar op.  Max
over classes + the baseline's (F+25)/64 decode give the argmax with
first-index tie-break.

Data parallel over 8 cores (64 patch rows each); per half (8 of 16
column chunks) the layout matches the old kernel: partition p = w mod
128 (16 patch-cols x 8 pixels), free n = r*512 + wcl*64 + prow.
"""

import numpy as np
import ml_dtypes

import concourse.bass as bass
import concourse.mybir as mybir
import concourse.tile as tile
from concourse import bacc
from concourse.bass_utils import run_bass_kernel_spmd

NCORES = 8
B, H, W = 4, 1024, 2048
DSF = 8
NCLS = 20
GH, GW = H // DSF, W // DSF  # 128, 256 output grid
ROWS = (B * H) // NCORES     # 512 label rows per core
PROWS = ROWS // DSF          # 64 patch rows per core
P = 128
WC = W // P                  # 16 column chunks of 128
HALVES = 2
WCH = WC // HALVES           # 8 chunks per half
FREE = WC * ROWS             # 8192
HFREE = FREE // HALVES       # 4096
NPL = 5                      # packed planes (4 classes each)

_DT = mybir.dt
_A = mybir.AluOpType
_AF = mybir.ActivationFunctionType

# Set by test.py to request a traced run.
TRACE = False
LAST_RESULTS = None


def _aux_arrays(class_weights: np.ndarray):
    """Host-built constants: stage-A/recovery lhsT, scalar APs, biases."""
    w = np.asarray(class_weights, dtype=np.float32)

    # stage-A lhsT: plane a sums 8-partition groups into M-slot a*16 + j.
    lhA = np.zeros((P, NPL * P), dtype=np.float32)
    for a in range(NPL):
        for p in range(P):
            lhA[p, a * P + a * 16 + p // 8] = 1.0
    lhA = lhA.astype(ml_dtypes.bfloat16)

    # recovery lhsT: E3[m = a*16+j] = 64*w[4a+3]*(d3(a) - sum_k d_k(a+1)) -
    # (4a+3).  Blocks k=0..3 contract the 80 digit partitions (K=80);
    # block 4 is the bias row (K=1, contracted against a ones tile).
    lhR = np.zeros((P, 5 * P), dtype=np.float32)
    for a in range(NPL):
        c = 4 * a + 3
        for j in range(16):
            m = a * 16 + j
            lhR[a * 16 + j, 3 * P + m] = 64.0 * w[c]
            if a + 1 < NPL:
                for k in range(4):
                    lhR[(a + 1) * 16 + j, k * P + m] = -64.0 * w[c]
            lhR[0, 4 * P + m] = -float(c)
    lhR = lhR.astype(ml_dtypes.bfloat16)

    # per-partition scalars for direct classes: E_jd = 64*w[4a+jd]*d - c
    wap = np.zeros((P, 3), dtype=np.float32)
    cap = np.zeros((P, 3), dtype=np.float32)
    for jd in range(3):
        for m in range(NPL * 16):
            a = m // 16
            wap[m, jd] = 64.0 * w[4 * a + jd]
            cap[m, jd] = float(4 * a + jd)

    # ACT biases: [-0.492 (t3/t2), -0.498 (d1), -0.375 (d0), 0.0]
    actb = np.zeros((P, 4), dtype=np.float32)
    actb[:, 0] = -0.492
    actb[:, 1] = -0.498
    actb[:, 2] = -0.375
    return lhA, lhR, wap, cap, actb


def _build():
    nc = bacc.Bacc(
        "TRN2",
        target_bir_lowering=False,
        debug=False,
        num_devices=NCORES,
    )
    u_d = nc.dram_tensor("u", [P, FREE], _DT.int16, kind="ExternalInput").ap()
    lha_d = nc.dram_tensor("lha", [P, NPL * P], _DT.bfloat16, kind="ExternalInput").ap()
    lhr_d = nc.dram_tensor("lhr", [P, 5 * P], _DT.bfloat16, kind="ExternalInput").ap()
    wap_d = nc.dram_tensor("wap", [P, 3], _DT.float32, kind="ExternalInput").ap()
    cap_d = nc.dram_tensor("cap", [P, 3], _DT.float32, kind="ExternalInput").ap()
    actb_d = nc.dram_tensor("actb", [P, 4], _DT.float32, kind="ExternalInput").ap()
    out_d = nc.dram_tensor("out", [16, HALVES * 512], _DT.int32, kind="ExternalOutput").ap()

    with tile.TileContext(nc) as tc:
        with (
            tc.tile_pool(name="const", bufs=1) as cpool,
            tc.tile_pool(name="u", bufs=1) as upool,
            tc.tile_pool(name="pk", bufs=2) as kpool,
            tc.tile_pool(name="psA", bufs=2, space="PSUM") as pApool,
            tc.tile_pool(name="psE", bufs=2, space="PSUM") as pEpool,
            tc.tile_pool(name="tail", bufs=2) as tpool,
            tc.tile_pool(name="outp", bufs=1) as outpool,
        ):
            # u chunks own the sync queue from t=0; consts ride SWDGE
            ut = upool.tile([P, FREE], _DT.int16)
            NCH = FREE // 2048
            for ch in range(NCH):
                nc.sync.dma_start(out=ut[:, ch * 2048:(ch + 1) * 2048],
                                  in_=u_d[:, ch * 2048:(ch + 1) * 2048])

            lhA = cpool.tile([P, NPL * P], _DT.bfloat16)
            nc.gpsimd.dma_start(out=lhA[:], in_=lha_d)
            lhR = cpool.tile([P, 5 * P], _DT.bfloat16)
            nc.gpsimd.dma_start(out=lhR[:], in_=lhr_d)
            wap = cpool.tile([P, 3], _DT.float32)
            nc.gpsimd.dma_start(out=wap[:], in_=wap_d)
            cap = cpool.tile([P, 3], _DT.float32)
            nc.gpsimd.dma_start(out=cap[:], in_=cap_d)
            actb = cpool.tile([P, 4], _DT.float32)
            nc.gpsimd.dma_start(out=actb[:], in_=actb_d)
            ones = cpool.tile([1, 512], _DT.bfloat16)
            nc.vector.memset(ones[:, :], 1.0)

            out_t = outpool.tile([16, HALVES * 512], _DT.int32)

            banks = []
            for hf in range(HALVES):
                base = hf * HFREE
                bank = pApool.tile([P, 512], _DT.float32, name=f"S{hf}", tag=f"S{hf}")
                banks.append(bank)
                for a in range(NPL):
                    pk = kpool.tile([P, HFREE], _DT.int16, name=f"pk{a}", tag=f"pk{a}")
                    nchunk = 2 if (hf == 0 and a == 0) else 1
                    for c in range(nchunk):
                        w0, w1 = c * HFREE // nchunk, (c + 1) * HFREE // nchunk
                        nc.vector.tensor_scalar(
                            out=pk[:, w0:w1],
                            in0=ut[:, base + w0:base + w1],
                            scalar1=float((145 + 24 * a) * 128),
                            scalar2=float(3072 * a),
                            op0=_A.min, op1=_A.subtract,
                        )
                        rhs = pk[:].bitcast(_DT.bfloat16)
                        for r in range(w0 // 512, w1 // 512):
                            nc.tensor.matmul(
                                bank[:, :],
                                lhA[:, a * P:(a + 1) * P],
                                rhs[:, r * 512:(r + 1) * 512],
                                start=(a == 0 and r == 0),
                                stop=(a == NPL - 1 and r == DSF - 1),
                            )

            for hf in range(HALVES):
                bank = banks[hf]
                hp = tc.high_priority() if hf == HALVES - 1 else None
                if hp is not None:
                    hp.__enter__()

                # digit peel, reading PSUM directly
                s80 = bank[0:80, :]
                t3 = tpool.tile([80, 512], _DT.int16, name="t3", tag="t3")
                nc.scalar.activation(t3[:], s80, _AF.Identity,
                                     bias=actb[0:80, 0:1], scale=1.0 / 262144)
                t2 = tpool.tile([80, 512], _DT.int16, name="t2", tag="t2")
                nc.scalar.activation(t2[:], s80, _AF.Identity,
                                     bias=actb[0:80, 0:1], scale=1.0 / 4096)
                d2b = tpool.tile([80, 512], _DT.bfloat16, name="d2b", tag="d2b")
                nc.vector.scalar_tensor_tensor(
                    out=d2b[:], in0=t3[:], scalar=-64.0, in1=t2[:],
                    op0=_A.mult, op1=_A.add)
                rem2 = tpool.tile([80, 512], _DT.float32, name="rem2", tag="rem2")
                nc.vector.scalar_tensor_tensor(
                    out=rem2[:], in0=t2[:], scalar=-4096.0, in1=s80,
                    op0=_A.mult, op1=_A.add)
                d1i = tpool.tile([80, 512], _DT.int16, name="d1i", tag="d1i")
                nc.scalar.activation(d1i[:], rem2[:], _AF.Identity,
                                     bias=actb[0:80, 1:2], scale=1.0 / 64)
                d0f = tpool.tile([80, 512], _DT.float32, name="d0f", tag="d0f")
                nc.vector.scalar_tensor_tensor(
                    out=d0f[:], in0=d1i[:], scalar=-64.0, in1=rem2[:],
                    op0=_A.mult, op1=_A.add)
                d0i = tpool.tile([80, 512], _DT.int16, name="d0i", tag="d0i")
                nc.scalar.activation(d0i[:], d0f[:], _AF.Identity,
                                     bias=actb[0:80, 2:3], scale=1.0)

                # bf16 digit tiles: d3b off-path on ACT, d1b/d0b on DVE 4x
                d3b = tpool.tile([80, 512], _DT.bfloat16, name="d3b", tag="d3b")
                nc.scalar.activation(d3b[:], t3[:], _AF.Identity,
                                     bias=actb[0:80, 3:4], scale=1.0)
                d1b = tpool.tile([80, 512], _DT.bfloat16, name="d1b", tag="d1b")
                nc.vector.tensor_scalar(out=d1b[:], in0=d1i[:], scalar1=0.0,
                                        scalar2=None, op0=_A.add)
                d0b = tpool.tile([80, 512], _DT.bfloat16, name="d0b", tag="d0b")
                nc.vector.tensor_scalar(out=d0b[:], in0=d0i[:], scalar1=0.0,
                                        scalar2=None, op0=_A.add)

                # E for chain classes: bias row + digits in reverse order so
                # the last-ready digit (d0b) is the final accumulate
                ps2 = pEpool.tile([P, 512], _DT.float32, name=f"E{hf}", tag=f"E{hf}")
                dbs = (d0b, d1b, d2b, d3b)
                for k in (3, 2, 1, 0):
                    nc.tensor.matmul(
                        ps2[:, :], lhR[0:80, k * P:(k + 1) * P], dbs[k][:, :],
                        start=(k == 3), stop=False,
                    )
                nc.tensor.matmul(
                    ps2[:, :], lhR[0:1, 4 * P:5 * P], ones[:, :],
                    start=False, stop=True,
                )

                # E for direct classes: per-partition scalars on GPSIMD
                es = []
                for jd, db in ((0, d0b), (1, d1b), (2, d2b)):
                    e = tpool.tile([80, 512], _DT.float32, name=f"e{jd}", tag=f"e{jd}")
                    nc.gpsimd.tensor_scalar(
                        out=e[:], in0=db[:],
                        scalar1=wap[0:80, jd:jd + 1], scalar2=cap[0:80, jd:jd + 1],
                        op0=_A.mult, op1=_A.subtract,
                    )
                    es.append(e)

                # tree max over the 4 E sources
                m01 = tpool.tile([80, 512], _DT.float32, name="m01", tag="m01")
                nc.vector.tensor_tensor(out=m01[:], in0=es[0][:], in1=es[1][:],
                                        op=_A.max)
                m23 = tpool.tile([80, 512], _DT.float32, name="m23", tag="m23")
                nc.vector.tensor_tensor(out=m23[:], in0=es[2][:], in1=ps2[0:80, :],
                                        op=_A.max)
                m3 = tpool.tile([96, 512], _DT.float32, name="m3", tag="m3")
                nc.vector.tensor_tensor(out=m3[0:80, :], in0=m01[:], in1=m23[:],
                                        op=_A.max)

                # fold the 5 plane-partitions (a*16+j) down to 16 (j)
                if hf == HALVES - 1:
                    # exposed tail: partition moves via DVE stream_shuffle
                    # (low latency) instead of DMA round-trips
                    idm = list(range(32))
                    upm = [16 + (i % 16) for i in range(32)]
                    sh1 = tpool.tile([32, 512], _DT.float32, name="sh1", tag="sh1")
                    nc.vector.stream_shuffle(sh1[:], m3[32:64, :], idm)
                    sh3 = tpool.tile([32, 512], _DT.float32, name="sh3", tag="sh3")
                    nc.vector.stream_shuffle(sh3[:], m3[64:96, :], idm)
                    f1 = tpool.tile([32, 512], _DT.float32, name="f1", tag="f1")
                    nc.vector.tensor_tensor(out=f1[:], in0=m3[0:32, :], in1=sh1[:],
                                            op=_A.max)
                    sh2 = tpool.tile([32, 512], _DT.float32, name="sh2", tag="sh2")
                    nc.vector.stream_shuffle(sh2[:], f1[:], upm)
                    f2 = tpool.tile([16, 512], _DT.float32, name="f2", tag="f2")
                    nc.vector.tensor_tensor(out=f2[:], in0=f1[0:16, :], in1=sh2[0:16, :],
                                            op=_A.max)
                    f3 = tpool.tile([16, 512], _DT.float32, name="f3", tag="f3")
                    nc.vector.tensor_tensor(out=f3[:], in0=f2[:], in1=sh3[0:16, :],
                                            op=_A.max)
                else:
                    t1 = tpool.tile([32, 512], _DT.float32, name="t1", tag="t1")
                    nc.sync.dma_start(out=t1[:], in_=m3[32:64, :])
                    t3f = tpool.tile([16, 512], _DT.float32, name="t3f", tag="t3f")
                    nc.sync.dma_start(out=t3f[:], in_=m3[64:80, :])
                    f1 = tpool.tile([32, 512], _DT.float32, name="f1", tag="f1")
                    nc.vector.tensor_tensor(out=f1[:], in0=m3[0:32, :], in1=t1[:],
                                            op=_A.max)
                    t2f = tpool.tile([16, 512], _DT.float32, name="t2f", tag="t2f")
                    nc.sync.dma_start(out=t2f[:], in_=f1[16:32, :])
                    f2 = tpool.tile([16, 512], _DT.float32, name="f2", tag="f2")
                    nc.vector.tensor_tensor(out=f2[:], in0=f1[0:16, :], in1=t2f[:],
                                            op=_A.max)
                    f3 = tpool.tile([16, 512], _DT.float32, name="f3", tag="f3")
                    nc.vector.tensor_tensor(out=f3[:], in0=f2[:], in1=t3f[:],
                                            op=_A.max)

                # decode: F = 64*w*n - c; W = cast((F + 25)/64); c* = 64W - F
                wi = tpool.tile([16, 512], _DT.int32, name="wi", tag="wi")
                wi_eng = nc.vector if hf == HALVES - 1 else nc.gpsimd
                wi_eng.tensor_scalar(
                    out=wi[:], in0=f3[:],
                    scalar1=25.0, scalar2=1.0 / 64.0,
                    op0=_A.add, op1=_A.mult,
                )
                nc.vector.scalar_tensor_tensor(
                    out=out_t[:, hf * 512:(hf + 1) * 512], in0=wi[:],
                    scalar=64.0, in1=f3[:],
                    op0=_A.mult, op1=_A.subtract,
                )
                nc.sync.dma_start(
                    out=out_d[:, hf * 512:(hf + 1) * 512],
                    in_=out_t[:, hf * 512:(hf + 1) * 512],
                )
                if hp is not None:
                    hp.__exit__(None, None, None)
    nc.finalize()
    return nc


_CACHED = None


def _get_nc():
    global _CACHED
    if _CACHED is None:
        _CACHED = _build()
    return _CACHED


def kernel(labels: np.ndarray, class_weights: np.ndarray, dsf) -> np.ndarray:
    global LAST_RESULTS
    dsf = int(np.asarray(dsf))
    assert dsf == DSF, f"kernel hardcodes dsf=8, got {dsf}"
    labels = np.asarray(labels)
    out_dtype = labels.dtype
    cw = np.asarray(class_weights, dtype=np.float32)

    # host prep: shard rows, encode u16 = (6x+127)<<7, transpose to
    # [p, hf, r, wcl, prow] (identical layout to the bf16 baseline)
    lab = labels.reshape(B * H, W).astype(np.int16)
    u_all = ((6 * lab + 127) << 7).astype(np.int16)
    lhA, lhR, wap, cap, actb = _aux_arrays(cw)
    in_maps = []
    for k in range(NCORES):
        shard = u_all[k * ROWS:(k + 1) * ROWS]                # [512, 2048]
        u = shard.reshape(PROWS, DSF, HALVES, WCH, P).transpose(4, 2, 1, 3, 0)
        u = np.ascontiguousarray(u).reshape(P, FREE)
        in_maps.append({
            "u": u,
            "lha": lhA,
            "lhr": lhR,
            "wap": wap,
            "cap": cap,
            "actb": actb,
        })

    nc = _get_nc()
    res = run_bass_kernel_spmd(
        nc, in_maps, core_ids=list(range(NCORES)), trace=TRACE,
    )
    LAST_RESULTS = res

    # unshard: core k out [16, 1024] int32; out[jj, hf*512 + wcl*64 + prow]
    # -> modes[patch_row = 64k + prow, j = (hf*8 + wcl)*16 + jj]
    modes = np.empty((B * GH, GW), dtype=np.int64)
    for k in range(NCORES):
        o = res.results[k]["out"].reshape(16, HALVES, WCH, PROWS)
        blk = o.transpose(3, 1, 2, 0).reshape(PROWS, WC * 16)
        modes[k * PROWS:(k + 1) * PROWS] = blk
    return modes.reshape(B, GH, GW).astype(out_dtype)
